# revision 45
# baseline (speedup 1.0000x reference)
"""GNN attention layer (gnn_message_passing) on 8 TRN2 NeuronCores.

Strategy (dst-sharded):
  - Core k owns the 6250 nodes whose total-degree rank ≡ k (mod 8); within a
    core, nodes are sorted by in-degree descending so each 128-node tile has a
    near-uniform degree K_t (exact-degree tiling: ~2% slot padding).
  - Device: projection z = W^T @ Xcat^T per local shard (W stationary on
    TensorE, PE-transpose back to node-major), AllGather the z table, then per
    tile: dma_gather neighbor z rows (4 SWDGE queues round-robin: desc-gen
    runs on all 4 Q7 core pairs in parallel), dot products + segment softmax +
    weighted aggregation on VectorE/ScalarE, batched ELU, one output DMA.
  - Gather index space: signed int16 with the table base at row 32768, so the
    whole 50176-row z table is addressable in one call (desc-gen address math
    is unsigned-stride x signed-idx).
  - Host scatters per-core outputs back to original node order.
"""
import sys

import numpy as np

sys.path.insert(0, '/opt/trn_rl_repo')

N = 50000
D = 256
F = 64
NC = 8
NPC = N // NC            # 6250
P = 128
RPT = 127                # real nodes per tile; row 127 is always a pad so the
                         # last gather position of every call is idx 0 (>=0,
                         # never hits the ucode trailing-negative trim)
NT = (NPC + RPT - 1) // RPT  # 50
NODES_PAD = NT * P       # 6400
VTOT = NC * NODES_PAD    # 51200
MID = 32768              # signed-idx base row of the z table
SLOPE = 0.2
NEG = -1e9
GCOLS = 16               # gather call width: 16 cols = 2048 idxs

_BUILD_CACHE = {}


# --------------------------------------------------------------------------
# host-side preprocessing
# --------------------------------------------------------------------------

def _wrap_cols(grid_cols):
    """[P, C] slot grid columns -> wrapped idx layout [128, 8*C] int16.

    gather position g = j*128 + p  ->  wrapped[g%16, g//16], replicated x8
    (one replica per SWDGE queue's 32-partition group).
    """
    lst = grid_cols.T.reshape(-1)            # column-major
    w = lst.reshape(-1, 16).T.astype(np.int16)
    return np.tile(w, (8, 1))


def _preprocess(d_sim, m_sim, W_d, W_m, node_type, src, dst):
    src = np.asarray(src).astype(np.int64)
    dst = np.asarray(dst).astype(np.int64)
    node_type = np.asarray(node_type)
    d_sim = np.asarray(d_sim, np.float32)
    m_sim = np.asarray(m_sim, np.float32)

    deg_all = np.bincount(dst, minlength=N)
    # round-robin node->core assignment by total-degree rank: all cores get
    # near-identical degree profiles, so the cross-core max-K penalty is tiny
    rank = np.argsort(np.argsort(deg_all, kind='stable'), kind='stable')
    owner = (rank % NC).astype(np.int64)

    # position i (0..NPC-1) within a core -> device row (row 127 of each
    # 128-row tile is reserved as a pad row)
    devrow_of_pos = (np.arange(NPC) // RPT) * P + (np.arange(NPC) % RPT)

    pos_of = np.empty(N, np.int64)     # node -> device row within its core
    perms = []                         # core -> node ids by position
    Ks = np.zeros((NC, NT), np.int64)
    for k in range(NC):
        nodes_k = np.where(owner == k)[0]
        assert len(nodes_k) == NPC
        order = np.argsort(-deg_all[nodes_k], kind='stable')
        perm0 = nodes_k[order]
        pos_of[perm0] = devrow_of_pos
        perms.append(perm0)
        dk = np.zeros(NODES_PAD, np.int64)
        dk[devrow_of_pos] = deg_all[perm0]
        Ks[k] = dk.reshape(NT, P).max(1)

    K = Ks.max(0)                      # shared per-tile slot count
    SK = int(K.sum())
    zpos = owner * NODES_PAD + pos_of  # node -> z table row

    Wcat = np.concatenate([np.asarray(W_d, np.float32),
                           np.asarray(W_m, np.float32)], 0)

    # process tiles smallest-K first: DVE warmup keeps pace with the gathers
    # (big tiles first caused a 24us gather stall + 74us DVE tail)
    tile_order = sorted(range(NT), key=lambda t: int(K[t]))

    # shared call program: per tile, ceil(K/16) calls of <=16 columns
    calls = []                         # (tile, col0, cw)
    for t in tile_order:
        kt = int(K[t])
        c0 = 0
        while c0 < kt:
            cw = min(GCOLS, kt - c0)
            calls.append((t, c0, cw))
            c0 += cw

    per_core = []
    iot = np.arange(P, dtype=np.int64)
    for k in range(NC):
        perm0 = perms[k]
        sel = owner[dst] == k
        d_pos = pos_of[dst[sel]]
        vals = zpos[src[sel]] - MID    # signed idx into the z table
        o = np.argsort(d_pos, kind='stable')
        r = d_pos[o]
        v = vals[o]
        starts = np.searchsorted(r, np.arange(NODES_PAD))
        slot = np.arange(len(r)) - starts[r]

        deg_pad = np.zeros(NODES_PAD, np.int64)
        deg_pad[devrow_of_pos] = deg_all[perm0]
        idx_blocks = {}
        bias_blocks = {}
        for t in range(NT):
            kt = int(K[t])
            if kt == 0:
                continue
            grid = np.zeros((P, kt), np.int64)
            m = (r >= t * P) & (r < (t + 1) * P)
            grid[r[m] - t * P, slot[m]] = v[m]
            dega = deg_pad[t * P:(t + 1) * P]
            bias_blocks[t] = (
                (iot[None, :kt] >= dega[:, None]) * np.float32(NEG))
            idx_blocks[t] = grid

        idxw = np.zeros((128, 8 * SK), np.int16)
        ioff = 0
        for (t, c0, cw) in calls:
            idxw[:, ioff:ioff + 8 * cw] = _wrap_cols(
                idx_blocks[t][:, c0:c0 + cw])
            ioff += 8 * cw
        assert ioff == 8 * SK
        biasw = np.concatenate(
            [bias_blocks[t] for t in tile_order if int(K[t]) > 0],
            1).astype(np.float32)
        assert biasw.shape[1] == SK

        perm_pad = np.full(NODES_PAD, -1, np.int64)
        perm_pad[devrow_of_pos] = perm0
        ids = perm0
        t_mask = (node_type[ids] == 1).astype(np.float32)
        xcatT = np.zeros((2 * D, NODES_PAD), np.float32)
        xcatT[:D, devrow_of_pos] = (d_sim[ids] * t_mask[:, None]).T
        xcatT[D:, devrow_of_pos] = (m_sim[ids] * (1.0 - t_mask)[:, None]).T

        per_core.append(dict(perm=perm_pad, xcatT=xcatT, idxw=idxw,
                             biasw=biasw))

    meta = dict(K=K, SK=SK, deg_all=deg_all, Wcat=Wcat, calls=calls,
                tile_order=tile_order)
    return per_core, meta


# --------------------------------------------------------------------------
# custom DVE ops (registered into concourse.dve_ops at build time)
# --------------------------------------------------------------------------

def _register_dve_ops():
    from concourse import dve_ops
    from concourse.dve_spec import (Spec, Src0, Src1, C0, C2, scan, AluOp,
                                    maxx, lower, _has_src1)
    from concourse.dve_uop import DveOpSpec

    if 'DOT_SCAN_GNN' in dve_ops._SUB_OPCODE_FOR_NAME:
        return
    specs = {
        # out[p, j] = cumsum_j(in0 * in1) over the whole free stream
        'DOT_SCAN_GNN': Spec(
            body=scan(AluOp.ADD, Src0 * Src1),
            reference=lambda in0, in1: np.cumsum(
                (in0 * in1).reshape(in0.shape[0], -1),
                axis=1).reshape(in0.shape)),
        # out = max(in0*imm2, in0) + in1 ; accum_out = max(out)
        'LRELU_BIAS_MAX_GNN': Spec(
            body=maxx(Src0 * C2, Src0) + Src1, accum=AluOp.MAX,
            reference=lambda in0, in1, imm2: (
                np.maximum(in0 * imm2, in0) + in1,
                (np.maximum(in0 * imm2, in0) + in1).max(-1, keepdims=True))),
        # out = (in0 + in1) * s0
        'ADD_SCALE_GNN': Spec(
            body=(Src0 + Src1) * C0,
            reference=lambda in0, in1, s0: (in0 + in1) * s0),
    }
    for name, sp in specs.items():
        row = 1 + len(dve_ops.OPS)
        shas = {}
        for ver in ('v3', 'v4'):
            u = lower(sp, ver=ver)
            d = DveOpSpec(name=name, opcode=row, uops=u,
                          rd1_en=_has_src1(sp))
            shas[ver] = d.sha(ver)
        op = dve_ops.DveOp(name, sp, subdim=False, uops_sha=shas)
        dve_ops.OPS.append(op)
        dve_ops.CUSTOM_DVE_SPECS[name] = sp
        dve_ops._SUB_OPCODE_FOR_NAME[name] = row
        setattr(dve_ops, name, op)


# --------------------------------------------------------------------------
# device program
# --------------------------------------------------------------------------

def _build(K, SK):
    import concourse.bass as bass
    import concourse.mybir as mybir
    import concourse.tile as tile
    import concourse.bacc as bacc

    _register_dve_ops()
    from concourse.dve_ops import (DOT_SCAN_GNN, LRELU_BIAS_MAX_GNN,
                                   ADD_SCALE_GNN)

    f32 = mybir.dt.float32
    i16 = mybir.dt.int16
    Alu = mybir.AluOpType
    Act = mybir.ActivationFunctionType

    nc = bacc.Bacc('TRN2', target_bir_lowering=False, debug=False,
                   num_devices=NC, num_swdge_queues=4)
    xcatT_d = nc.dram_tensor('xcatT', [2 * D, NODES_PAD], f32,
                             kind='ExternalInput')
    wcat_d = nc.dram_tensor('wcat', [2 * D, F], f32, kind='ExternalInput')
    ident_d = nc.dram_tensor('ident', [F, F], f32, kind='ExternalInput')
    idxw_d = nc.dram_tensor('idxw', [128, 8 * SK], i16, kind='ExternalInput')
    biasw_d = nc.dram_tensor('biasw', [128, SK], f32, kind='ExternalInput')
    h_d = nc.dram_tensor('h', [NODES_PAD, F], f32, kind='ExternalOutput')

    def bcast_mid(ap, n):
        """[128, W] AP -> [128, n, W] with 0-step middle axis."""
        return bass.AP(ap.tensor, ap.offset, [ap.ap[0], [0, n], ap.ap[1]])

    KMAX = int(max(K))

    with tile.TileContext(nc) as tc:
        with tc.tile_pool(name='dram', bufs=1, space='DRAM') as dram, \
             tc.tile_pool(name='persist', bufs=1) as sb1, \
             tc.tile_pool(name='xt', bufs=4) as sbx, \
             tc.tile_pool(name='zt', bufs=2) as sbzt, \
             tc.tile_pool(name='g', bufs=5) as sbg, \
             tc.tile_pool(name='t', bufs=4) as sbt, \
             tc.tile_pool(name='small', bufs=4) as sbs, \
             tc.tile_pool(name='pz', bufs=2, space='PSUM') as pz, \
             tc.tile_pool(name='pt', bufs=4, space='PSUM') as pt:

            z_local = dram.tile([NODES_PAD, F], f32)
            z_all = dram.tile([VTOT, F], f32, addr_space='Shared')

            wcat_sb = sb1.tile([128, 4 * F], f32)
            for c in range(4):
                nc.sync.dma_start(out=wcat_sb[:, c * F:(c + 1) * F],
                                  in_=wcat_d[c * 128:(c + 1) * 128, :])
            ident_sb = sb1.tile([F, F], f32)
            nc.sync.dma_start(out=ident_sb[:], in_=ident_d[:])
            idx_sb = sb1.tile([128, 8 * SK], i16)
            nc.gpsimd.dma_start(out=idx_sb[:], in_=idxw_d[:])
            bias_sb = sb1.tile([128, SK], f32)
            nc.gpsimd.dma_start(out=bias_sb[:], in_=biasw_d[:])

            zstore = sb1.tile([128, NT * F], f32)
            hstore = sb1.tile([128, NT * F], f32)
            den_all = sb1.tile([128, NT], f32)

            # ---- phase 1: projection (W stationary, PE transpose back) ----
            CH = [(i * 512, 512) for i in range(12)] + [(12 * 512, 256)]
            for off, w in CH:
                ps = pz.tile([64, 512], f32, tag='zT')
                for c in range(4):
                    xt = sbx.tile([128, 512], f32, tag='xt')
                    nc.sync.dma_start(
                        out=xt[:, :w],
                        in_=xcatT_d[c * 128:(c + 1) * 128, off:off + w])
                    nc.tensor.matmul(out=ps[:, :w],
                                     lhsT=wcat_sb[:, c * F:(c + 1) * F],
                                     rhs=xt[:, :w],
                                     start=(c == 0), stop=(c == 3))
                zTs = sbzt.tile([64, 512], f32, tag='zTs')
                nc.scalar.copy(zTs[:, :w], ps[:, :w])
                for j in range(w // 128):
                    t = off // 128 + j
                    tp = pt.tile([128, F], f32, tag='tp')
                    nc.tensor.transpose(tp[:], zTs[:, j * 128:(j + 1) * 128],
                                        ident_sb[:])
                    zt = zstore[:, t * F:(t + 1) * F]
                    nc.scalar.copy(zt, tp[:])
                    nc.sync.dma_start(out=z_local[t * 128:(t + 1) * 128, :],
                                      in_=zt)

            # ---- AllGather z ----
            nc.gpsimd.collective_compute(
                'AllGather', Alu.bypass,
                replica_groups=[list(range(NC))],
                ins=[z_local[:]], outs=[z_all[:]])

            z_base = z_all[MID:, :]

            # ---- phase 2: edges ----
            ioff = 0
            boff = 0
            qload = [0] * 4   # greedy per-queue idx balancing: round-robin
                              # leaves 55% skew (full calls pile on 2 queues)
            tile_order = sorted(range(NT), key=lambda tt: int(K[tt]))
            for t in tile_order:
                kt = int(K[t])
                if kt == 0:
                    continue
                g = sbg.tile([128, kt * F], f32, tag='g')
                c0 = 0
                while c0 < kt:
                    cw = min(GCOLS, kt - c0)
                    nidx = 128 * cw
                    q = min(range(4), key=lambda i: qload[i])
                    qload[q] += nidx
                    nc.gpsimd.dma_gather(
                        g[:, c0 * F:(c0 + cw) * F].rearrange(
                            'p (k f) -> p k f', f=F),
                        z_base,
                        idx_sb[:, ioff:ioff + 8 * cw],
                        nidx, nidx, F, single_packet=False,
                        queue_num=q)
                    c0 += cw
                    ioff += 8 * cw
                zt = zstore[:, t * F:(t + 1) * F]
                # fused dot products: cumsum of g*z over the (k,f) stream,
                # then e_k = S[(k+1)F] - S[kF] (with S[0]=0 seed column)
                stmp = sbt.tile([128, kt * F + 1], f32, tag='tmp')
                nc.scalar.memzero(stmp[:, :1])
                nc.vector._custom_dve(
                    DOT_SCAN_GNN,
                    out=stmp[:, 1:].rearrange('p (k f) -> p k f', f=F),
                    in0=g[:].rearrange('p (k f) -> p k f', f=F),
                    in1=bcast_mid(zt, kt))
                sa = stmp[:]
                s_hi = bass.AP(sa.tensor, sa.offset + F, [sa.ap[0], [F, kt]])
                s_lo = bass.AP(sa.tensor, sa.offset, [sa.ap[0], [F, kt]])
                e = sbs.tile([128, kt], f32, tag='e')
                nc.vector.tensor_tensor(out=e[:], in0=s_hi, in1=s_lo,
                                        op=Alu.subtract)
                # fused leaky_relu + pad-mask bias + row max
                m = sbs.tile([128, 1], f32, tag='m')
                nc.vector._custom_dve(
                    LRELU_BIAS_MAX_GNN, out=e[:], in0=e[:],
                    in1=bias_sb[:, boff:boff + kt], imm2=SLOPE,
                    accum_out=m[:])
                boff += kt
                nm = sbs.tile([128, 1], f32, tag='nm')
                nc.scalar.mul(nm[:], m[:], -1.0)
                ex = sbs.tile([128, kt], f32, tag='ex')
                nc.scalar.activation(out=ex[:], in_=e[:], func=Act.Exp,
                                     bias=nm[:, :1], scale=1.0,
                                     accum_out=den_all[:, t:t + 1])
                # tmp2[p, k, f] = g[p, k, f] * ex[p, k]  (all contiguous)
                tmp2 = sbt.tile([128, kt * F], f32, tag='tmp')
                exa = ex[:]
                ex_b = bass.AP(exa.tensor, exa.offset,
                               [exa.ap[0], exa.ap[1], [0, F]])
                nc.vector.tensor_tensor(
                    out=tmp2[:].rearrange('p (k f) -> p k f', f=F),
                    in0=g[:].rearrange('p (k f) -> p k f', f=F),
                    in1=ex_b, op=Alu.mult)
                # halving-tree reduce over k (unnormalized; 1/den applied in
                # one batched pass at the end)
                h = kt
                if h == 1:
                    nc.scalar.copy(hstore[:, t * F:(t + 1) * F], tmp2[:, :F])
                while h > 1:
                    hi = h // 2
                    lo = h - hi
                    out_ap = (hstore[:, t * F:(t + 1) * F] if h == 2
                              else tmp2[:, :hi * F])
                    nc.vector.tensor_tensor(
                        out=out_ap, in0=tmp2[:, :hi * F],
                        in1=tmp2[:, lo * F:(lo + hi) * F], op=Alu.add)
                    h = lo

            # ---- tail: batched 1/den scale + ELU + one output DMA ----
            rec_all = sb1.tile([128, NT], f32)
            nc.vector.reciprocal(out=rec_all[:], in_=den_all[:])
            ra = rec_all[:]
            rec_b = bass.AP(ra.tensor, ra.offset, [ra.ap[0], ra.ap[1], [0, F]])
            nc.vector.tensor_tensor(
                out=hstore[:].rearrange('p (t f) -> p t f', f=F),
                in0=hstore[:].rearrange('p (t f) -> p t f', f=F),
                in1=rec_b, op=Alu.mult)
            hv = hstore[:]
            hn = sb1.tile([128, NT * F], f32)
            nc.vector.tensor_scalar(out=hn[:], in0=hv, scalar1=0.0,
                                    scalar2=None, op0=Alu.min)
            nc.scalar.activation(out=hn[:], in_=hn[:], func=Act.Exp)
            nc.vector.tensor_scalar(out=hv, in0=hv, scalar1=0.0,
                                    scalar2=None, op0=Alu.max)
            nc.vector.scalar_tensor_tensor(
                out=hv, in0=hn[:], scalar=-1.0, in1=hv,
                op0=Alu.add, op1=Alu.add)
            h_ap = h_d[:]
            h_out = bass.AP(h_ap.tensor, h_ap.offset,
                            [[F, 128], [128 * F, NT], [1, F]])
            nc.sync.dma_start(
                out=h_out,
                in_=hstore[:].rearrange('p (t f) -> p t f', f=F))

    nc.compile()
    return nc


# --------------------------------------------------------------------------
# entry point
# --------------------------------------------------------------------------

def kernel(d_sim, m_sim, W_d, W_m, node_type, src, dst):
    from concourse.bass_utils import run_bass_kernel_spmd

    per_core, meta = _preprocess(d_sim, m_sim, W_d, W_m, node_type, src, dst)
    key = tuple(meta['K'])
    if key not in _BUILD_CACHE:
        _BUILD_CACHE[key] = _build(meta['K'], meta['SK'])
    nc = _BUILD_CACHE[key]

    wcat = meta['Wcat']
    ident = np.eye(F, dtype=np.float32)
    in_maps = [{'xcatT': pc['xcatT'], 'wcat': wcat, 'ident': ident,
                'idxw': pc['idxw'], 'biasw': pc['biasw']} for pc in per_core]
    res = run_bass_kernel_spmd(nc, in_maps, core_ids=list(range(NC)),
                               trace=False)

    h_full = np.zeros((N, F), np.float32)
    for k, pc in enumerate(per_core):
        sel = pc['perm'] >= 0
        h_full[pc['perm'][sel]] = res.results[k]['h'][sel]
    h_full[meta['deg_all'] == 0] = 0.0
    return h_full


# revision 46
# speedup vs baseline: 1.0577x; 1.0577x over previous
"""GNN attention layer (gnn_message_passing) on 8 TRN2 NeuronCores.

Strategy (dst-sharded):
  - Core k owns the 6250 nodes whose total-degree rank ≡ k (mod 8); within a
    core, nodes are sorted by in-degree descending so each 128-node tile has a
    near-uniform degree K_t (exact-degree tiling: ~2% slot padding).
  - Device: projection z = W^T @ Xcat^T per local shard (W stationary on
    TensorE, PE-transpose back to node-major), AllGather the z table, then per
    tile: dma_gather neighbor z rows (4 SWDGE queues round-robin: desc-gen
    runs on all 4 Q7 core pairs in parallel), dot products + segment softmax +
    weighted aggregation on VectorE/ScalarE, batched ELU, one output DMA.
  - Gather index space: signed int16 with the table base at row 32768, so the
    whole 50176-row z table is addressable in one call (desc-gen address math
    is unsigned-stride x signed-idx).
  - Host scatters per-core outputs back to original node order.
"""
import sys

import numpy as np

sys.path.insert(0, '/opt/trn_rl_repo')

N = 50000
D = 256
F = 64
NC = 8
NPC = N // NC            # 6250
P = 128
RPT = 127                # real nodes per tile; row 127 is always a pad so the
                         # last gather position of every call is idx 0 (>=0,
                         # never hits the ucode trailing-negative trim)
NT = (NPC + RPT - 1) // RPT  # 50
NODES_PAD = NT * P       # 6400
VTOT = NC * NODES_PAD    # 51200
MID = 32768              # signed-idx base row of the z table
SLOPE = 0.2
NEG = -1e9
GCOLS = 16               # gather call width: 16 cols = 2048 idxs

_BUILD_CACHE = {}


# --------------------------------------------------------------------------
# host-side preprocessing
# --------------------------------------------------------------------------

def _wrap_cols(grid_cols):
    """[P, C] slot grid columns -> wrapped idx layout [128, 8*C] int16.

    gather position g = j*128 + p  ->  wrapped[g%16, g//16], replicated x8
    (one replica per SWDGE queue's 32-partition group).
    """
    lst = grid_cols.T.reshape(-1)            # column-major
    w = lst.reshape(-1, 16).T.astype(np.int16)
    return np.tile(w, (8, 1))


def _preprocess(d_sim, m_sim, W_d, W_m, node_type, src, dst):
    src = np.asarray(src).astype(np.int64)
    dst = np.asarray(dst).astype(np.int64)
    node_type = np.asarray(node_type)
    d_sim = np.asarray(d_sim, np.float32)
    m_sim = np.asarray(m_sim, np.float32)

    deg_all = np.bincount(dst, minlength=N)
    # round-robin node->core assignment by total-degree rank: all cores get
    # near-identical degree profiles, so the cross-core max-K penalty is tiny
    rank = np.argsort(np.argsort(deg_all, kind='stable'), kind='stable')
    owner = (rank % NC).astype(np.int64)

    # position i (0..NPC-1) within a core -> device row (row 127 of each
    # 128-row tile is reserved as a pad row)
    devrow_of_pos = (np.arange(NPC) // RPT) * P + (np.arange(NPC) % RPT)

    pos_of = np.empty(N, np.int64)     # node -> device row within its core
    perms = []                         # core -> node ids by position
    Ks = np.zeros((NC, NT), np.int64)
    for k in range(NC):
        nodes_k = np.where(owner == k)[0]
        assert len(nodes_k) == NPC
        order = np.argsort(-deg_all[nodes_k], kind='stable')
        perm0 = nodes_k[order]
        pos_of[perm0] = devrow_of_pos
        perms.append(perm0)
        dk = np.zeros(NODES_PAD, np.int64)
        dk[devrow_of_pos] = deg_all[perm0]
        Ks[k] = dk.reshape(NT, P).max(1)

    K = Ks.max(0)                      # shared per-tile slot count
    SK = int(K.sum())
    zpos = owner * NODES_PAD + pos_of  # node -> z table row

    Wcat = np.concatenate([np.asarray(W_d, np.float32),
                           np.asarray(W_m, np.float32)], 0)

    # process tiles smallest-K first: DVE warmup keeps pace with the gathers
    # (big tiles first caused a 24us gather stall + 74us DVE tail)
    tile_order = list(range(NT))

    # shared call program: per tile, ceil(K/16) calls of <=16 columns
    calls = []                         # (tile, col0, cw)
    for t in tile_order:
        kt = int(K[t])
        c0 = 0
        while c0 < kt:
            cw = min(GCOLS, kt - c0)
            calls.append((t, c0, cw))
            c0 += cw

    per_core = []
    iot = np.arange(P, dtype=np.int64)
    for k in range(NC):
        perm0 = perms[k]
        sel = owner[dst] == k
        d_pos = pos_of[dst[sel]]
        vals = zpos[src[sel]] - MID    # signed idx into the z table
        o = np.argsort(d_pos, kind='stable')
        r = d_pos[o]
        v = vals[o]
        starts = np.searchsorted(r, np.arange(NODES_PAD))
        slot = np.arange(len(r)) - starts[r]

        deg_pad = np.zeros(NODES_PAD, np.int64)
        deg_pad[devrow_of_pos] = deg_all[perm0]
        idx_blocks = {}
        bias_blocks = {}
        for t in range(NT):
            kt = int(K[t])
            if kt == 0:
                continue
            grid = np.zeros((P, kt), np.int64)
            m = (r >= t * P) & (r < (t + 1) * P)
            grid[r[m] - t * P, slot[m]] = v[m]
            dega = deg_pad[t * P:(t + 1) * P]
            bias_blocks[t] = (
                (iot[None, :kt] >= dega[:, None]) * np.float32(NEG))
            idx_blocks[t] = grid

        idxw = np.zeros((128, 8 * SK), np.int16)
        ioff = 0
        for (t, c0, cw) in calls:
            idxw[:, ioff:ioff + 8 * cw] = _wrap_cols(
                idx_blocks[t][:, c0:c0 + cw])
            ioff += 8 * cw
        assert ioff == 8 * SK
        biasw = np.concatenate(
            [bias_blocks[t] for t in tile_order if int(K[t]) > 0],
            1).astype(np.float32)
        assert biasw.shape[1] == SK

        perm_pad = np.full(NODES_PAD, -1, np.int64)
        perm_pad[devrow_of_pos] = perm0
        ids = perm0
        t_mask = (node_type[ids] == 1).astype(np.float32)
        xcatT = np.zeros((2 * D, NODES_PAD), np.float32)
        xcatT[:D, devrow_of_pos] = (d_sim[ids] * t_mask[:, None]).T
        xcatT[D:, devrow_of_pos] = (m_sim[ids] * (1.0 - t_mask)[:, None]).T

        per_core.append(dict(perm=perm_pad, xcatT=xcatT, idxw=idxw,
                             biasw=biasw))

    meta = dict(K=K, SK=SK, deg_all=deg_all, Wcat=Wcat, calls=calls,
                tile_order=tile_order)
    return per_core, meta


# --------------------------------------------------------------------------
# custom DVE ops (registered into concourse.dve_ops at build time)
# --------------------------------------------------------------------------

def _register_dve_ops():
    from concourse import dve_ops
    from concourse.dve_spec import (Spec, Src0, Src1, C0, C2, scan, AluOp,
                                    maxx, lower, _has_src1)
    from concourse.dve_uop import DveOpSpec

    if 'DOT_SCAN_GNN' in dve_ops._SUB_OPCODE_FOR_NAME:
        return
    specs = {
        # out[p, j] = cumsum_j(in0 * in1) over the whole free stream
        'DOT_SCAN_GNN': Spec(
            body=scan(AluOp.ADD, Src0 * Src1),
            reference=lambda in0, in1: np.cumsum(
                (in0 * in1).reshape(in0.shape[0], -1),
                axis=1).reshape(in0.shape)),
        # out = max(in0*imm2, in0) + in1 ; accum_out = max(out)
        'LRELU_BIAS_MAX_GNN': Spec(
            body=maxx(Src0 * C2, Src0) + Src1, accum=AluOp.MAX,
            reference=lambda in0, in1, imm2: (
                np.maximum(in0 * imm2, in0) + in1,
                (np.maximum(in0 * imm2, in0) + in1).max(-1, keepdims=True))),
        # out = (in0 + in1) * s0
        'ADD_SCALE_GNN': Spec(
            body=(Src0 + Src1) * C0,
            reference=lambda in0, in1, s0: (in0 + in1) * s0),
    }
    for name, sp in specs.items():
        row = 1 + len(dve_ops.OPS)
        shas = {}
        for ver in ('v3', 'v4'):
            u = lower(sp, ver=ver)
            d = DveOpSpec(name=name, opcode=row, uops=u,
                          rd1_en=_has_src1(sp))
            shas[ver] = d.sha(ver)
        op = dve_ops.DveOp(name, sp, subdim=False, uops_sha=shas)
        dve_ops.OPS.append(op)
        dve_ops.CUSTOM_DVE_SPECS[name] = sp
        dve_ops._SUB_OPCODE_FOR_NAME[name] = row
        setattr(dve_ops, name, op)


# --------------------------------------------------------------------------
# device program
# --------------------------------------------------------------------------

def _build(K, SK):
    import concourse.bass as bass
    import concourse.mybir as mybir
    import concourse.tile as tile
    import concourse.bacc as bacc

    _register_dve_ops()
    from concourse.dve_ops import (DOT_SCAN_GNN, LRELU_BIAS_MAX_GNN,
                                   ADD_SCALE_GNN)

    f32 = mybir.dt.float32
    i16 = mybir.dt.int16
    Alu = mybir.AluOpType
    Act = mybir.ActivationFunctionType

    nc = bacc.Bacc('TRN2', target_bir_lowering=False, debug=False,
                   num_devices=NC, num_swdge_queues=4)
    xcatT_d = nc.dram_tensor('xcatT', [2 * D, NODES_PAD], f32,
                             kind='ExternalInput')
    wcat_d = nc.dram_tensor('wcat', [2 * D, F], f32, kind='ExternalInput')
    ident_d = nc.dram_tensor('ident', [F, F], f32, kind='ExternalInput')
    idxw_d = nc.dram_tensor('idxw', [128, 8 * SK], i16, kind='ExternalInput')
    biasw_d = nc.dram_tensor('biasw', [128, SK], f32, kind='ExternalInput')
    h_d = nc.dram_tensor('h', [NODES_PAD, F], f32, kind='ExternalOutput')

    def bcast_mid(ap, n):
        """[128, W] AP -> [128, n, W] with 0-step middle axis."""
        return bass.AP(ap.tensor, ap.offset, [ap.ap[0], [0, n], ap.ap[1]])

    KMAX = int(max(K))

    with tile.TileContext(nc) as tc:
        with tc.tile_pool(name='dram', bufs=1, space='DRAM') as dram, \
             tc.tile_pool(name='persist', bufs=1) as sb1, \
             tc.tile_pool(name='xt', bufs=4) as sbx, \
             tc.tile_pool(name='zt', bufs=2) as sbzt, \
             tc.tile_pool(name='g', bufs=5) as sbg, \
             tc.tile_pool(name='t', bufs=4) as sbt, \
             tc.tile_pool(name='small', bufs=4) as sbs, \
             tc.tile_pool(name='pz', bufs=2, space='PSUM') as pz, \
             tc.tile_pool(name='pt', bufs=4, space='PSUM') as pt:

            z_local = dram.tile([NODES_PAD, F], f32)
            z_all = dram.tile([VTOT, F], f32, addr_space='Shared')

            wcat_sb = sb1.tile([128, 4 * F], f32)
            for c in range(4):
                nc.sync.dma_start(out=wcat_sb[:, c * F:(c + 1) * F],
                                  in_=wcat_d[c * 128:(c + 1) * 128, :])
            ident_sb = sb1.tile([F, F], f32)
            nc.sync.dma_start(out=ident_sb[:], in_=ident_d[:])
            idx_sb = sb1.tile([128, 8 * SK], i16)
            nc.gpsimd.dma_start(out=idx_sb[:], in_=idxw_d[:])
            bias_sb = sb1.tile([128, SK], f32)
            nc.gpsimd.dma_start(out=bias_sb[:], in_=biasw_d[:])

            zstore = sb1.tile([128, NT * F], f32)
            hstore = sb1.tile([128, NT * F], f32)
            den_all = sb1.tile([128, NT], f32)

            # ---- phase 1: projection (W stationary, PE transpose back) ----
            CH = [(i * 512, 512) for i in range(12)] + [(12 * 512, 256)]
            for off, w in CH:
                ps = pz.tile([64, 512], f32, tag='zT')
                for c in range(4):
                    xt = sbx.tile([128, 512], f32, tag='xt')
                    nc.sync.dma_start(
                        out=xt[:, :w],
                        in_=xcatT_d[c * 128:(c + 1) * 128, off:off + w])
                    nc.tensor.matmul(out=ps[:, :w],
                                     lhsT=wcat_sb[:, c * F:(c + 1) * F],
                                     rhs=xt[:, :w],
                                     start=(c == 0), stop=(c == 3))
                zTs = sbzt.tile([64, 512], f32, tag='zTs')
                nc.scalar.copy(zTs[:, :w], ps[:, :w])
                for j in range(w // 128):
                    t = off // 128 + j
                    tp = pt.tile([128, F], f32, tag='tp')
                    nc.tensor.transpose(tp[:], zTs[:, j * 128:(j + 1) * 128],
                                        ident_sb[:])
                    zt = zstore[:, t * F:(t + 1) * F]
                    nc.scalar.copy(zt, tp[:])
                    nc.sync.dma_start(out=z_local[t * 128:(t + 1) * 128, :],
                                      in_=zt)

            # ---- AllGather z ----
            nc.gpsimd.collective_compute(
                'AllGather', Alu.bypass,
                replica_groups=[list(range(NC))],
                ins=[z_local[:]], outs=[z_all[:]])

            z_base = z_all[MID:, :]

            # ---- phase 2: edges ----
            ioff = 0
            boff = 0
            qload = [0] * 4   # greedy per-queue idx balancing: round-robin
                              # leaves 55% skew (full calls pile on 2 queues)
            tile_order = list(range(NT))
            rec_all = sb1.tile([128, NT], f32)
            hn = sb1.tile([128, NT * F], f32)
            h_ap = h_d[:]

            def _finish_half(t0, t1):
                # batched 1/den scale + ELU + output DMA for tiles [t0, t1)
                n = t1 - t0
                nc.vector.reciprocal(out=rec_all[:, t0:t1],
                                     in_=den_all[:, t0:t1])
                ra = rec_all[:, t0:t1]
                rec_b = bass.AP(ra.tensor, ra.offset,
                                [ra.ap[0], ra.ap[1], [0, F]])
                hv = hstore[:, t0 * F:t1 * F]
                nc.vector.tensor_tensor(
                    out=hv.rearrange('p (t f) -> p t f', f=F),
                    in0=hv.rearrange('p (t f) -> p t f', f=F),
                    in1=rec_b, op=Alu.mult)
                hnv = hn[:, t0 * F:t1 * F]
                nc.vector.tensor_scalar(out=hnv, in0=hv, scalar1=0.0,
                                        scalar2=None, op0=Alu.min)
                nc.scalar.activation(out=hnv, in_=hnv, func=Act.Exp)
                nc.vector.tensor_scalar(out=hv, in0=hv, scalar1=0.0,
                                        scalar2=None, op0=Alu.max)
                nc.vector.scalar_tensor_tensor(
                    out=hv, in0=hnv, scalar=-1.0, in1=hv,
                    op0=Alu.add, op1=Alu.add)
                h_out = bass.AP(h_ap.tensor, h_ap.offset + t0 * 128 * F,
                                [[F, 128], [128 * F, n], [1, F]])
                nc.sync.dma_start(
                    out=h_out, in_=hv.rearrange('p (t f) -> p t f', f=F))

            HALF = NT // 2
            done_tiles = 0
            for t in tile_order:
                kt = int(K[t])
                if kt == 0:
                    continue
                g = sbg.tile([128, kt * F], f32, tag='g')
                c0 = 0
                while c0 < kt:
                    cw = min(GCOLS, kt - c0)
                    nidx = 128 * cw
                    q = min(range(4), key=lambda i: qload[i])
                    qload[q] += nidx
                    nc.gpsimd.dma_gather(
                        g[:, c0 * F:(c0 + cw) * F].rearrange(
                            'p (k f) -> p k f', f=F),
                        z_base,
                        idx_sb[:, ioff:ioff + 8 * cw],
                        nidx, nidx, F, single_packet=False,
                        queue_num=q)
                    c0 += cw
                    ioff += 8 * cw
                zt = zstore[:, t * F:(t + 1) * F]
                # fused dot products: cumsum of g*z over the (k,f) stream,
                # then e_k = S[(k+1)F] - S[kF] (with S[0]=0 seed column)
                stmp = sbt.tile([128, kt * F + 1], f32, tag='tmp')
                nc.scalar.memzero(stmp[:, :1])
                nc.vector._custom_dve(
                    DOT_SCAN_GNN,
                    out=stmp[:, 1:].rearrange('p (k f) -> p k f', f=F),
                    in0=g[:].rearrange('p (k f) -> p k f', f=F),
                    in1=bcast_mid(zt, kt))
                sa = stmp[:]
                s_hi = bass.AP(sa.tensor, sa.offset + F, [sa.ap[0], [F, kt]])
                s_lo = bass.AP(sa.tensor, sa.offset, [sa.ap[0], [F, kt]])
                e = sbs.tile([128, kt], f32, tag='e')
                nc.vector.tensor_tensor(out=e[:], in0=s_hi, in1=s_lo,
                                        op=Alu.subtract)
                # fused leaky_relu + pad-mask bias + row max
                m = sbs.tile([128, 1], f32, tag='m')
                nc.vector._custom_dve(
                    LRELU_BIAS_MAX_GNN, out=e[:], in0=e[:],
                    in1=bias_sb[:, boff:boff + kt], imm2=SLOPE,
                    accum_out=m[:])
                boff += kt
                nm = sbs.tile([128, 1], f32, tag='nm')
                nc.scalar.mul(nm[:], m[:], -1.0)
                ex = sbs.tile([128, kt], f32, tag='ex')
                nc.scalar.activation(out=ex[:], in_=e[:], func=Act.Exp,
                                     bias=nm[:, :1], scale=1.0,
                                     accum_out=den_all[:, t:t + 1])
                # tmp2[p, k, f] = g[p, k, f] * ex[p, k]  (all contiguous)
                tmp2 = sbt.tile([128, kt * F], f32, tag='tmp')
                exa = ex[:]
                ex_b = bass.AP(exa.tensor, exa.offset,
                               [exa.ap[0], exa.ap[1], [0, F]])
                nc.vector.tensor_tensor(
                    out=tmp2[:].rearrange('p (k f) -> p k f', f=F),
                    in0=g[:].rearrange('p (k f) -> p k f', f=F),
                    in1=ex_b, op=Alu.mult)
                # halving-tree reduce over k (unnormalized; 1/den applied in
                # one batched pass at the end)
                h = kt
                if h == 1:
                    nc.scalar.copy(hstore[:, t * F:(t + 1) * F], tmp2[:, :F])
                while h > 1:
                    hi = h // 2
                    lo = h - hi
                    out_ap = (hstore[:, t * F:(t + 1) * F] if h == 2
                              else tmp2[:, :hi * F])
                    nc.vector.tensor_tensor(
                        out=out_ap, in0=tmp2[:, :hi * F],
                        in1=tmp2[:, lo * F:(lo + hi) * F], op=Alu.add)
                    h = lo
                done_tiles += 1
                if done_tiles == HALF:
                    _finish_half(0, HALF)

            _finish_half(HALF, NT)

    nc.compile()
    return nc


# --------------------------------------------------------------------------
# entry point
# --------------------------------------------------------------------------

def kernel(d_sim, m_sim, W_d, W_m, node_type, src, dst):
    from concourse.bass_utils import run_bass_kernel_spmd

    per_core, meta = _preprocess(d_sim, m_sim, W_d, W_m, node_type, src, dst)
    key = tuple(meta['K'])
    if key not in _BUILD_CACHE:
        _BUILD_CACHE[key] = _build(meta['K'], meta['SK'])
    nc = _BUILD_CACHE[key]

    wcat = meta['Wcat']
    ident = np.eye(F, dtype=np.float32)
    in_maps = [{'xcatT': pc['xcatT'], 'wcat': wcat, 'ident': ident,
                'idxw': pc['idxw'], 'biasw': pc['biasw']} for pc in per_core]
    res = run_bass_kernel_spmd(nc, in_maps, core_ids=list(range(NC)),
                               trace=False)

    h_full = np.zeros((N, F), np.float32)
    for k, pc in enumerate(per_core):
        sel = pc['perm'] >= 0
        h_full[pc['perm'][sel]] = res.results[k]['h'][sel]
    h_full[meta['deg_all'] == 0] = 0.0
    return h_full


# revision 47
# speedup vs baseline: 1.0663x; 1.0081x over previous
"""GNN attention layer (gnn_message_passing) on 8 TRN2 NeuronCores.

Strategy (dst-sharded):
  - Core k owns the 6250 nodes whose total-degree rank ≡ k (mod 8); within a
    core, nodes are sorted by in-degree descending so each 128-node tile has a
    near-uniform degree K_t (exact-degree tiling: ~2% slot padding).
  - Device: projection z = W^T @ Xcat^T per local shard (W stationary on
    TensorE, PE-transpose back to node-major), AllGather the z table, then per
    tile: dma_gather neighbor z rows (4 SWDGE queues round-robin: desc-gen
    runs on all 4 Q7 core pairs in parallel), dot products + segment softmax +
    weighted aggregation on VectorE/ScalarE, batched ELU, one output DMA.
  - Gather index space: signed int16 with the table base at row 32768, so the
    whole 50176-row z table is addressable in one call (desc-gen address math
    is unsigned-stride x signed-idx).
  - Host scatters per-core outputs back to original node order.
"""
import sys

import numpy as np

sys.path.insert(0, '/opt/trn_rl_repo')

N = 50000
D = 256
F = 64
NC = 8
NPC = N // NC            # 6250
P = 128
RPT = 127                # real nodes per tile; row 127 is always a pad so the
                         # last gather position of every call is idx 0 (>=0,
                         # never hits the ucode trailing-negative trim)
NT = (NPC + RPT - 1) // RPT  # 50
NODES_PAD = NT * P       # 6400
VTOT = NC * NODES_PAD    # 51200
MID = 32768              # signed-idx base row of the z table
SLOPE = 0.2
NEG = -1e9
GCOLS = 16               # gather call width: 16 cols = 2048 idxs

_BUILD_CACHE = {}


# --------------------------------------------------------------------------
# host-side preprocessing
# --------------------------------------------------------------------------

def _wrap_cols(grid_cols):
    """[P, C] slot grid columns -> wrapped idx layout [128, 8*C] int16.

    gather position g = j*128 + p  ->  wrapped[g%16, g//16], replicated x8
    (one replica per SWDGE queue's 32-partition group).
    """
    lst = grid_cols.T.reshape(-1)            # column-major
    w = lst.reshape(-1, 16).T.astype(np.int16)
    return np.tile(w, (8, 1))


def _preprocess(d_sim, m_sim, W_d, W_m, node_type, src, dst):
    src = np.asarray(src).astype(np.int64)
    dst = np.asarray(dst).astype(np.int64)
    node_type = np.asarray(node_type)
    d_sim = np.asarray(d_sim, np.float32)
    m_sim = np.asarray(m_sim, np.float32)

    deg_all = np.bincount(dst, minlength=N)
    # round-robin node->core assignment by total-degree rank: all cores get
    # near-identical degree profiles, so the cross-core max-K penalty is tiny
    rank = np.argsort(np.argsort(deg_all, kind='stable'), kind='stable')
    owner = (rank % NC).astype(np.int64)

    # position i (0..NPC-1) within a core -> device row (row 127 of each
    # 128-row tile is reserved as a pad row)
    devrow_of_pos = (np.arange(NPC) // RPT) * P + (np.arange(NPC) % RPT)

    pos_of = np.empty(N, np.int64)     # node -> device row within its core
    perms = []                         # core -> node ids by position
    Ks = np.zeros((NC, NT), np.int64)
    for k in range(NC):
        nodes_k = np.where(owner == k)[0]
        assert len(nodes_k) == NPC
        order = np.argsort(-deg_all[nodes_k], kind='stable')
        perm0 = nodes_k[order]
        pos_of[perm0] = devrow_of_pos
        perms.append(perm0)
        dk = np.zeros(NODES_PAD, np.int64)
        dk[devrow_of_pos] = deg_all[perm0]
        Ks[k] = dk.reshape(NT, P).max(1)

    K = Ks.max(0)                      # shared per-tile slot count
    SK = int(K.sum())
    zpos = owner * NODES_PAD + pos_of  # node -> z table row

    Wcat = np.concatenate([np.asarray(W_d, np.float32),
                           np.asarray(W_m, np.float32)], 0)

    # process tiles smallest-K first: DVE warmup keeps pace with the gathers
    # (big tiles first caused a 24us gather stall + 74us DVE tail)
    tile_order = list(range(NT))

    # shared call program: per tile, ceil(K/16) calls of <=16 columns
    calls = []                         # (tile, col0, cw)
    for t in tile_order:
        kt = int(K[t])
        c0 = 0
        while c0 < kt:
            cw = min(GCOLS, kt - c0)
            calls.append((t, c0, cw))
            c0 += cw

    per_core = []
    iot = np.arange(P, dtype=np.int64)
    for k in range(NC):
        perm0 = perms[k]
        sel = owner[dst] == k
        d_pos = pos_of[dst[sel]]
        vals = zpos[src[sel]] - MID    # signed idx into the z table
        o = np.argsort(d_pos, kind='stable')
        r = d_pos[o]
        v = vals[o]
        starts = np.searchsorted(r, np.arange(NODES_PAD))
        slot = np.arange(len(r)) - starts[r]

        deg_pad = np.zeros(NODES_PAD, np.int64)
        deg_pad[devrow_of_pos] = deg_all[perm0]
        idx_blocks = {}
        bias_blocks = {}
        for t in range(NT):
            kt = int(K[t])
            if kt == 0:
                continue
            grid = np.zeros((P, kt), np.int64)
            m = (r >= t * P) & (r < (t + 1) * P)
            grid[r[m] - t * P, slot[m]] = v[m]
            dega = deg_pad[t * P:(t + 1) * P]
            bias_blocks[t] = (
                (iot[None, :kt] >= dega[:, None]) * np.float32(NEG))
            idx_blocks[t] = grid

        idxw = np.zeros((128, 8 * SK), np.int16)
        ioff = 0
        for (t, c0, cw) in calls:
            idxw[:, ioff:ioff + 8 * cw] = _wrap_cols(
                idx_blocks[t][:, c0:c0 + cw])
            ioff += 8 * cw
        assert ioff == 8 * SK
        biasw = np.concatenate(
            [bias_blocks[t] for t in tile_order if int(K[t]) > 0],
            1).astype(np.float32)
        assert biasw.shape[1] == SK

        perm_pad = np.full(NODES_PAD, -1, np.int64)
        perm_pad[devrow_of_pos] = perm0
        ids = perm0
        t_mask = (node_type[ids] == 1).astype(np.float32)
        xcatT = np.zeros((2 * D, NODES_PAD), np.float32)
        xcatT[:D, devrow_of_pos] = (d_sim[ids] * t_mask[:, None]).T
        xcatT[D:, devrow_of_pos] = (m_sim[ids] * (1.0 - t_mask)[:, None]).T

        per_core.append(dict(perm=perm_pad, xcatT=xcatT, idxw=idxw,
                             biasw=biasw))

    meta = dict(K=K, SK=SK, deg_all=deg_all, Wcat=Wcat, calls=calls,
                tile_order=tile_order)
    return per_core, meta


# --------------------------------------------------------------------------
# custom DVE ops (registered into concourse.dve_ops at build time)
# --------------------------------------------------------------------------

def _register_dve_ops():
    from concourse import dve_ops
    from concourse.dve_spec import (Spec, Src0, Src1, C0, C2, scan, AluOp,
                                    maxx, lower, _has_src1)
    from concourse.dve_uop import DveOpSpec

    if 'DOT_SCAN_GNN' in dve_ops._SUB_OPCODE_FOR_NAME:
        return
    specs = {
        # out[p, j] = cumsum_j(in0 * in1) over the whole free stream
        'DOT_SCAN_GNN': Spec(
            body=scan(AluOp.ADD, Src0 * Src1),
            reference=lambda in0, in1: np.cumsum(
                (in0 * in1).reshape(in0.shape[0], -1),
                axis=1).reshape(in0.shape)),
        # out = max(in0*imm2, in0) + in1 ; accum_out = max(out)
        'LRELU_BIAS_MAX_GNN': Spec(
            body=maxx(Src0 * C2, Src0) + Src1, accum=AluOp.MAX,
            reference=lambda in0, in1, imm2: (
                np.maximum(in0 * imm2, in0) + in1,
                (np.maximum(in0 * imm2, in0) + in1).max(-1, keepdims=True))),
        # out = (in0 + in1) * s0
        'ADD_SCALE_GNN': Spec(
            body=(Src0 + Src1) * C0,
            reference=lambda in0, in1, s0: (in0 + in1) * s0),
    }
    for name, sp in specs.items():
        row = 1 + len(dve_ops.OPS)
        shas = {}
        for ver in ('v3', 'v4'):
            u = lower(sp, ver=ver)
            d = DveOpSpec(name=name, opcode=row, uops=u,
                          rd1_en=_has_src1(sp))
            shas[ver] = d.sha(ver)
        op = dve_ops.DveOp(name, sp, subdim=False, uops_sha=shas)
        dve_ops.OPS.append(op)
        dve_ops.CUSTOM_DVE_SPECS[name] = sp
        dve_ops._SUB_OPCODE_FOR_NAME[name] = row
        setattr(dve_ops, name, op)


# --------------------------------------------------------------------------
# device program
# --------------------------------------------------------------------------

def _build(K, SK):
    import concourse.bass as bass
    import concourse.mybir as mybir
    import concourse.tile as tile
    import concourse.bacc as bacc

    _register_dve_ops()
    from concourse.dve_ops import (DOT_SCAN_GNN, LRELU_BIAS_MAX_GNN,
                                   ADD_SCALE_GNN)

    f32 = mybir.dt.float32
    i16 = mybir.dt.int16
    Alu = mybir.AluOpType
    Act = mybir.ActivationFunctionType

    nc = bacc.Bacc('TRN2', target_bir_lowering=False, debug=False,
                   num_devices=NC, num_swdge_queues=4)
    xcatT_d = nc.dram_tensor('xcatT', [2 * D, NODES_PAD], f32,
                             kind='ExternalInput')
    wcat_d = nc.dram_tensor('wcat', [2 * D, F], f32, kind='ExternalInput')
    ident_d = nc.dram_tensor('ident', [F, F], f32, kind='ExternalInput')
    idxw_d = nc.dram_tensor('idxw', [128, 8 * SK], i16, kind='ExternalInput')
    biasw_d = nc.dram_tensor('biasw', [128, SK], f32, kind='ExternalInput')
    h_d = nc.dram_tensor('h', [NODES_PAD, F], f32, kind='ExternalOutput')

    def bcast_mid(ap, n):
        """[128, W] AP -> [128, n, W] with 0-step middle axis."""
        return bass.AP(ap.tensor, ap.offset, [ap.ap[0], [0, n], ap.ap[1]])

    KMAX = int(max(K))

    with tile.TileContext(nc) as tc:
        with tc.tile_pool(name='dram', bufs=1, space='DRAM') as dram, \
             tc.tile_pool(name='persist', bufs=1) as sb1, \
             tc.tile_pool(name='xt', bufs=4) as sbx, \
             tc.tile_pool(name='zt', bufs=2) as sbzt, \
             tc.tile_pool(name='g', bufs=5) as sbg, \
             tc.tile_pool(name='t', bufs=4) as sbt, \
             tc.tile_pool(name='small', bufs=4) as sbs, \
             tc.tile_pool(name='pz', bufs=2, space='PSUM') as pz, \
             tc.tile_pool(name='pt', bufs=4, space='PSUM') as pt:

            z_local = dram.tile([NODES_PAD, F], f32)
            z_all = dram.tile([VTOT, F], f32, addr_space='Shared')

            wcat_sb = sb1.tile([128, 4 * F], f32)
            for c in range(4):
                nc.sync.dma_start(out=wcat_sb[:, c * F:(c + 1) * F],
                                  in_=wcat_d[c * 128:(c + 1) * 128, :])
            ident_sb = sb1.tile([F, F], f32)
            nc.sync.dma_start(out=ident_sb[:], in_=ident_d[:])
            idx_sb = sb1.tile([128, 8 * SK], i16)
            nc.gpsimd.dma_start(out=idx_sb[:], in_=idxw_d[:])
            bias_sb = sb1.tile([128, SK], f32)
            nc.gpsimd.dma_start(out=bias_sb[:], in_=biasw_d[:])

            zstore = sb1.tile([128, NT * F], f32)
            hstore = sb1.tile([128, NT * F], f32)
            den_all = sb1.tile([128, NT], f32)

            # ---- phase 1: projection (W stationary, PE transpose back) ----
            CH = [(i * 512, 512) for i in range(12)] + [(12 * 512, 256)]
            for off, w in CH:
                ps = pz.tile([64, 512], f32, tag='zT')
                for c in range(4):
                    xt = sbx.tile([128, 512], f32, tag='xt')
                    nc.sync.dma_start(
                        out=xt[:, :w],
                        in_=xcatT_d[c * 128:(c + 1) * 128, off:off + w])
                    nc.tensor.matmul(out=ps[:, :w],
                                     lhsT=wcat_sb[:, c * F:(c + 1) * F],
                                     rhs=xt[:, :w],
                                     start=(c == 0), stop=(c == 3))
                zTs = sbzt.tile([64, 512], f32, tag='zTs')
                nc.scalar.copy(zTs[:, :w], ps[:, :w])
                for j in range(w // 128):
                    t = off // 128 + j
                    tp = pt.tile([128, F], f32, tag='tp')
                    nc.tensor.transpose(tp[:], zTs[:, j * 128:(j + 1) * 128],
                                        ident_sb[:])
                    zt = zstore[:, t * F:(t + 1) * F]
                    nc.scalar.copy(zt, tp[:])
                    nc.sync.dma_start(out=z_local[t * 128:(t + 1) * 128, :],
                                      in_=zt)

            # ---- AllGather z ----
            nc.gpsimd.collective_compute(
                'AllGather', Alu.bypass,
                replica_groups=[list(range(NC))],
                ins=[z_local[:]], outs=[z_all[:]])

            z_base = z_all[MID:, :]

            # ---- phase 2: edges ----
            ioff = 0
            boff = 0
            qload = [0] * 4   # greedy per-queue idx balancing: round-robin
                              # leaves 55% skew (full calls pile on 2 queues)
            tile_order = list(range(NT))
            for t in tile_order:
                kt = int(K[t])
                if kt == 0:
                    continue
                g = sbg.tile([128, kt * F], f32, tag='g')
                c0 = 0
                while c0 < kt:
                    cw = min(GCOLS, kt - c0)
                    nidx = 128 * cw
                    q = min(range(4), key=lambda i: qload[i])
                    qload[q] += nidx
                    nc.gpsimd.dma_gather(
                        g[:, c0 * F:(c0 + cw) * F].rearrange(
                            'p (k f) -> p k f', f=F),
                        z_base,
                        idx_sb[:, ioff:ioff + 8 * cw],
                        nidx, nidx, F, single_packet=False,
                        queue_num=q)
                    c0 += cw
                    ioff += 8 * cw
                zt = zstore[:, t * F:(t + 1) * F]
                # fused dot products: cumsum of g*z over the (k,f) stream,
                # then e_k = S[(k+1)F] - S[kF] (with S[0]=0 seed column)
                stmp = sbt.tile([128, kt * F + 1], f32, tag='tmp')
                nc.scalar.memzero(stmp[:, :1])
                nc.vector._custom_dve(
                    DOT_SCAN_GNN,
                    out=stmp[:, 1:].rearrange('p (k f) -> p k f', f=F),
                    in0=g[:].rearrange('p (k f) -> p k f', f=F),
                    in1=bcast_mid(zt, kt))
                sa = stmp[:]
                s_hi = bass.AP(sa.tensor, sa.offset + F, [sa.ap[0], [F, kt]])
                s_lo = bass.AP(sa.tensor, sa.offset, [sa.ap[0], [F, kt]])
                e = sbs.tile([128, kt], f32, tag='e')
                nc.vector.tensor_tensor(out=e[:], in0=s_hi, in1=s_lo,
                                        op=Alu.subtract)
                # fused leaky_relu + pad-mask bias + row max
                m = sbs.tile([128, 1], f32, tag='m')
                nc.vector._custom_dve(
                    LRELU_BIAS_MAX_GNN, out=e[:], in0=e[:],
                    in1=bias_sb[:, boff:boff + kt], imm2=SLOPE,
                    accum_out=m[:])
                boff += kt
                nm = sbs.tile([128, 1], f32, tag='nm')
                nc.scalar.mul(nm[:], m[:], -1.0)
                ex = sbs.tile([128, kt], f32, tag='ex')
                nc.scalar.activation(out=ex[:], in_=e[:], func=Act.Exp,
                                     bias=nm[:, :1], scale=1.0,
                                     accum_out=den_all[:, t:t + 1])
                # tmp2[p, k, f] = g[p, k, f] * ex[p, k]  (all contiguous)
                tmp2 = sbt.tile([128, kt * F], f32, tag='tmp')
                exa = ex[:]
                ex_b = bass.AP(exa.tensor, exa.offset,
                               [exa.ap[0], exa.ap[1], [0, F]])
                nc.vector.tensor_tensor(
                    out=tmp2[:].rearrange('p (k f) -> p k f', f=F),
                    in0=g[:].rearrange('p (k f) -> p k f', f=F),
                    in1=ex_b, op=Alu.mult)
                # halving-tree reduce over k (unnormalized; 1/den applied in
                # one batched pass at the end)
                h = kt
                if h == 1:
                    nc.scalar.copy(hstore[:, t * F:(t + 1) * F], tmp2[:, :F])
                while h > 1:
                    hi = h // 2
                    lo = h - hi
                    out_ap = (hstore[:, t * F:(t + 1) * F] if h == 2
                              else tmp2[:, :hi * F])
                    nc.vector.tensor_tensor(
                        out=out_ap, in0=tmp2[:, :hi * F],
                        in1=tmp2[:, lo * F:(lo + hi) * F], op=Alu.add)
                    h = lo

            # ---- tail: batched 1/den scale + ELU + one output DMA ----
            rec_all = sb1.tile([128, NT], f32)
            nc.vector.reciprocal(out=rec_all[:], in_=den_all[:])
            ra = rec_all[:]
            rec_b = bass.AP(ra.tensor, ra.offset, [ra.ap[0], ra.ap[1], [0, F]])
            nc.vector.tensor_tensor(
                out=hstore[:].rearrange('p (t f) -> p t f', f=F),
                in0=hstore[:].rearrange('p (t f) -> p t f', f=F),
                in1=rec_b, op=Alu.mult)
            hv = hstore[:]
            hn = sb1.tile([128, NT * F], f32)
            nc.vector.tensor_scalar(out=hn[:], in0=hv, scalar1=0.0,
                                    scalar2=None, op0=Alu.min)
            nc.scalar.activation(out=hn[:], in_=hn[:], func=Act.Exp)
            nc.vector.tensor_scalar(out=hv, in0=hv, scalar1=0.0,
                                    scalar2=None, op0=Alu.max)
            nc.vector.scalar_tensor_tensor(
                out=hv, in0=hn[:], scalar=-1.0, in1=hv,
                op0=Alu.add, op1=Alu.add)
            h_ap = h_d[:]
            h_out = bass.AP(h_ap.tensor, h_ap.offset,
                            [[F, 128], [128 * F, NT], [1, F]])
            nc.sync.dma_start(
                out=h_out,
                in_=hstore[:].rearrange('p (t f) -> p t f', f=F))

    nc.compile()
    return nc


# --------------------------------------------------------------------------
# entry point
# --------------------------------------------------------------------------

def kernel(d_sim, m_sim, W_d, W_m, node_type, src, dst):
    from concourse.bass_utils import run_bass_kernel_spmd

    per_core, meta = _preprocess(d_sim, m_sim, W_d, W_m, node_type, src, dst)
    key = tuple(meta['K'])
    if key not in _BUILD_CACHE:
        _BUILD_CACHE[key] = _build(meta['K'], meta['SK'])
    nc = _BUILD_CACHE[key]

    wcat = meta['Wcat']
    ident = np.eye(F, dtype=np.float32)
    in_maps = [{'xcatT': pc['xcatT'], 'wcat': wcat, 'ident': ident,
                'idxw': pc['idxw'], 'biasw': pc['biasw']} for pc in per_core]
    res = run_bass_kernel_spmd(nc, in_maps, core_ids=list(range(NC)),
                               trace=False)

    h_full = np.zeros((N, F), np.float32)
    for k, pc in enumerate(per_core):
        sel = pc['perm'] >= 0
        h_full[pc['perm'][sel]] = res.results[k]['h'][sel]
    h_full[meta['deg_all'] == 0] = 0.0
    return h_full
